# revision 12
# baseline (speedup 1.0000x reference)
"""Trainium2 Bass kernel for nn_Decoder (2-layer LSTM decoder + dot attention +
factorized embedding + summed cross-entropy), SPMD over 8 NeuronCores.

Sharding:
  - Scan: tensor-parallel over the 4H gate dim (512-wide shard per core,
    weights SBUF-resident); h0n/h1n AllGathered transposed each step (bf16).
    Attention batch-sharded (4 batches/core) with a batched softmax over all
    4 own batches (scores at psum partition strips 0/32/64/96); ctx
    AllGathered and landed in a (k c j) layout so the post-collective DMA is
    coarse-grained.
  - Output/CE: row-sharded (188 (t,b) rows per core), E^T streamed bf16,
    fused exp+row-sum via ACT accum_out; rowtile-0 vocab chunks interleaved
    into the AllGather bubbles of steps 33..46 to keep the PE warm.
  - Final loss AllReduce.
"""
import sys
sys.path.insert(0, '/opt/trn_rl_repo')

import numpy as np
import ml_dtypes
import concourse.bass as bass
import concourse.bacc as bacc
import concourse.tile as tile
import concourse.mybir as mybir
from concourse import bass_utils
from concourse.masks import make_identity

AF = mybir.ActivationFunctionType
ALU = mybir.AluOpType
AX = mybir.AxisListType
F32R = mybir.dt.float32r
F32 = mybir.dt.float32
BF16 = mybir.dt.bfloat16
I32 = mybir.dt.int32

NCORES = 8
S, B, C = 56, 32, 1024
T = 48
H = 1024
EMB = 512
RANK = 128
VOCAB = 32000
TS = T - 1
GS = 512
HS = 128
BS = B // NCORES
NROW = TS * BS
NCH = C // 128            # 8
NTOK = TS * B             # 1504
NTOKP = 1536              # padded to 12*128
VCHUNKS = [512] * 62 + [256]
assert sum(VCHUNKS) == VOCAB

DT4 = F32R
BF16NP = ml_dtypes.bfloat16


def build():
    nc = bacc.Bacc("TRN2", target_bir_lowering=False, debug=False,
                   num_devices=NCORES)

    def din(name, shape, dt):
        return nc.dram_tensor(name, shape, dt, kind="ExternalInput").ap()

    w0e = din("w0e", [EMB, GS], BF16)
    w0c = din("w0c", [C, GS], BF16)
    u0 = din("u0", [H, GS], BF16)
    w1 = din("w1", [H, GS], BF16)
    u1 = din("u1", [H, GS], BF16)
    b01 = din("b01", [1, 2 * GS], DT4)
    wo = din("wo", [H + C, EMB], BF16)
    boT = din("boT", [128, 4], F32)
    pmat = din("pmat", [RANK, EMB], DT4)
    pTb = din("pTb", [EMB, RANK], BF16)
    enc_own = din("enc_own", [S, BS * C], BF16)
    encT_own = din("encT_own", [128, NCH * BS * S], BF16)
    smask = din("smask", [128, S], F32)
    hT_init = din("hT_init", [2 * H, B], BF16)
    e_rows = din("e_rows", [VOCAB, RANK], F32)
    eT = din("eT", [RANK, VOCAB], BF16)
    tokidx = din("tokidx", [NTOKP, 1], I32)
    labidx = din("labidx", [NROW, 1], I32)
    vmask = din("vmask", [128, 2], F32)
    ones_in = din("ones_in", [128, B + 1], DT4)
    zctx = din("zctx", [128, NCH * B], BF16)

    out_loss = nc.dram_tensor("loss", [1, 1], F32, kind="ExternalOutput").ap()

    rg = [list(range(NCORES))]

    with tile.TileContext(nc, num_cores=NCORES) as tc:
        with tc.tile_pool(name="consts", bufs=1) as consts, \
             tc.tile_pool(name="wpool", bufs=1) as wpool, \
             tc.tile_pool(name="state", bufs=1) as state, \
             tc.tile_pool(name="dram", bufs=2, space="DRAM") as dram:

            id4 = consts.tile([128, 128], F32)
            make_identity(nc, id4)
            ones1 = consts.tile([1, B], DT4)
            nc.sync.dma_start(ones1[:], ones_in[0:1, :B])
            ones128 = consts.tile([128, 1], DT4)
            nc.sync.dma_start(ones128[:], ones_in[:, B:B + 1])

            def wtile(ap, shape, dt, tag):
                t_ = wpool.tile(shape, dt, tag=tag)
                nc.sync.dma_start(t_[:], ap[:])
                return t_

            def wtile_ch(ap, nchunk, width, dt, tag):
                """Load [nchunk*128, width] DRAM into [128, nchunk*width] SBUF."""
                t_ = wpool.tile([128, nchunk * width], dt, tag=tag)
                nc.sync.dma_start(
                    t_[:].rearrange("p (c g) -> p c g", c=nchunk),
                    ap[:].rearrange("(c p) g -> p c g", p=128))
                return t_

            w0e_s = wtile_ch(w0e, 4, GS, BF16, tag="w0e_s")
            w0c_s = wtile_ch(w0c, 8, GS, BF16, tag="w0c_s")
            u0_s = wtile_ch(u0, 8, GS, BF16, tag="u0_s")
            w1_s = wtile_ch(w1, 8, GS, BF16, tag="w1_s")
            u1_s = wtile_ch(u1, 8, GS, BF16, tag="u1_s")
            b01_s = wtile(b01, [1, 2 * GS], DT4, tag="b01_s")
            wo_s = wtile_ch(wo, 16, EMB, BF16, tag="wo_s")
            boT_s = wtile(boT, [128, 4], F32, tag="boT_s")
            pm_s = wtile(pmat, [RANK, EMB], DT4, tag="pm_s")
            pT_s = wtile_ch(pTb, 4, RANK, BF16, tag="pT_s")
            enc_s = wtile(enc_own, [S, BS * C], BF16, tag="enc_s")
            encT_s = wtile(encT_own, [128, NCH * BS * S], BF16, tag="encT_s")
            smask_s = wtile(smask, [128, S], F32, tag="smask_s")
            vmask_s = wtile(vmask, [128, 2], F32, tag="vmask_s")

            hT0 = state.tile([128, NCH * B], BF16)
            hT1 = state.tile([128, NCH * B], BF16)
            hT1own = state.tile([128, NCH * BS], BF16)
            ctxT = state.tile([128, NCH * B], BF16)
            ctxKin = state.tile([128, NCH * B], BF16)
            nc.sync.dma_start(ctxT[:], zctx[:])
            c0 = state.tile([B, HS], F32)
            nc.gpsimd.memset(c0[:], 0.0)
            c1 = state.tile([B, HS], F32)
            nc.gpsimd.memset(c1[:], 0.0)
            for ch in range(NCH):
                nc.sync.dma_start(hT0[:, ch * B:(ch + 1) * B],
                                  hT_init[ch * 128:(ch + 1) * 128, :])
                nc.sync.dma_start(hT1[:, ch * B:(ch + 1) * B],
                                  hT_init[H + ch * 128:H + (ch + 1) * 128, :])

            embT = state.tile([128, 4 * NTOKP], BF16)
            featsT = state.tile([128, 16 * NROW], BF16)
            elab = state.tile([128, 2 * RANK], F32)
            sum_e = state.tile([128, 2], F32)
            nc.gpsimd.memset(sum_e[:], 1.0)
            lab_ll = state.tile([128, 2], F32)
            nc.gpsimd.memset(lab_ll[:], 0.0)
            scmt = state.tile([128, S], F32)
            nc.gpsimd.memset(scmt[:], 0.0)

            pid = nc.partition_id()

            # ---------------- pre-phase: embT + label gathers ----------------
            with tc.tile_pool(name="pre_ps", bufs=2, space="PSUM") as pre_ps, \
                 tc.tile_pool(name="pre_sb", bufs=3) as pre_sb:
                egT = pre_sb.tile([128, NTOKP], DT4, tag="egT", bufs=1)
                idxall = pre_sb.tile([128, 12], I32, tag="idxall", bufs=1)
                nc.sync.dma_start(
                    idxall[:].rearrange("p (c o) -> p c o", o=1),
                    tokidx[:].rearrange("(c p) o -> p c o", p=128))
                for i in range(12):
                    r0 = i * 128
                    eg = pre_sb.tile([128, RANK], F32, tag="eg")
                    nc.gpsimd.indirect_dma_start(
                        out=eg[:], out_offset=None, in_=e_rows[:],
                        in_offset=bass.IndirectOffsetOnAxis(
                            ap=idxall[:, i:i + 1], axis=0))
                    ps = pre_ps.tile([128, 128], F32, tag="tr")
                    nc.tensor.transpose(ps[:, :], eg[:, :], id4[:, :])
                    nc.scalar.copy(egT[:, r0:r0 + 128], ps[:, :])
                for e in range(4):
                    for n0 in range(0, NTOKP, 512):
                        n1 = min(n0 + 512, NTOKP)
                        ps = pre_ps.tile([128, 512], F32, tag="mm")
                        nc.tensor.matmul(ps[:, :n1 - n0],
                                         pm_s[:, e * 128:(e + 1) * 128],
                                         egT[:, n0:n1], start=True, stop=True)
                        nc.scalar.copy(embT[:, e * NTOKP + n0:e * NTOKP + n1],
                                       ps[:, :n1 - n0])
                for rt, (r0, nr) in enumerate(((0, 128), (128, NROW - 128))):
                    idx = pre_sb.tile([128, 1], I32, tag="idx")
                    nc.sync.dma_start(idx[:nr], labidx[r0:r0 + nr, :])
                    nc.gpsimd.indirect_dma_start(
                        out=elab[:nr, rt * RANK:(rt + 1) * RANK],
                        out_offset=None, in_=e_rows[:],
                        in_offset=bass.IndirectOffsetOnAxis(ap=idx[:nr, :1], axis=0))

            # ---------------- scan + interleaved CE ----------------
            with tc.tile_pool(name="pg", bufs=2, space="PSUM") as pg, \
                 tc.tile_pool(name="ptr", bufs=1, space="PSUM") as ptr, \
                 tc.tile_pool(name="psc", bufs=1, space="PSUM") as psc, \
                 tc.tile_pool(name="pptp", bufs=1, space="PSUM") as pptp, \
                 tc.tile_pool(name="pcx", bufs=1, space="PSUM") as pcx, \
                 tc.tile_pool(name="plg", bufs=2, space="PSUM") as plg, \
                 tc.tile_pool(name="sb", bufs=3) as sb, \
                 tc.tile_pool(name="ebuf", bufs=8) as ebuf:

                def lstm_layer(gp, cstate, k):
                    sif = sb.tile([B, 256], F32, tag="sif")
                    nc.scalar.activation(sif[:], gp[:, 0:256], AF.Sigmoid)
                    so = sb.tile([B, HS], F32, tag="so")
                    nc.scalar.activation(so[:], gp[:, 384:512], AF.Sigmoid)
                    tg = sb.tile([B, HS], F32, tag="tg")
                    nc.scalar.activation(tg[:], gp[:, 256:384], AF.Tanh)
                    t1 = sb.tile([B, HS], F32, tag="t1")
                    nc.vector.tensor_mul(t1[:], sif[:, 128:256], cstate[:])
                    t2 = sb.tile([B, HS], F32, tag="t2")
                    nc.vector.tensor_mul(t2[:], sif[:, 0:128], tg[:])
                    nc.vector.tensor_add(cstate[:], t1[:], t2[:])
                    tch = sb.tile([B, HS], F32, tag="tch")
                    nc.scalar.activation(tch[:], cstate[:], AF.Tanh)
                    hn = sb.tile([B, HS], F32, tag=f"hn{k}")
                    nc.vector.tensor_mul(hn[:], so[:], tch[:])
                    return hn

                def gather_h(hn, dst, tag):
                    tp = ptr.tile([128, B], F32, tag="htr")
                    nc.tensor.transpose(tp[:], hn[:], id4[:B, :B])
                    stg = sb.tile([128, B], BF16, tag=f"stg{tag}")
                    nc.scalar.copy(stg[:], tp[:])
                    bi = dram.tile([128, B], BF16, tag=f"agi{tag}")
                    nc.sync.dma_start(bi[:], stg[:])
                    bo = dram.tile([128 * NCORES, B], BF16, tag=f"ago{tag}")
                    nc.gpsimd.collective_compute(
                        "AllGather", ALU.bypass, replica_groups=rg,
                        ins=[bi.opt()], outs=[bo.opt()])
                    for half in range(2):
                        nc.sync.dma_start(
                            dst[:].rearrange("p (c b) -> p c b", c=NCH)[
                                :, half * 4:(half + 1) * 4, :],
                            bo[:].rearrange("(c p) b -> p c b", p=128)[
                                :, half * 4:(half + 1) * 4, :])
                    return bo

                def ce_head(rt, r0, nr):
                    hpT = sb.tile([128, 4 * 128], BF16, tag="hpT")
                    for m in range(4):
                        ps = plg.tile([128, 512], F32, tag="lg")
                        for kk in range(16):
                            nc.tensor.matmul(
                                ps[:, :nr],
                                wo_s[:, kk * EMB + m * 128:kk * EMB + (m + 1) * 128],
                                featsT[:, kk * NROW + r0:kk * NROW + r0 + nr],
                                start=(kk == 0), stop=(kk == 15))
                        nc.scalar.activation(hpT[:, m * 128:m * 128 + nr],
                                             ps[:, :nr], AF.Tanh,
                                             bias=boT_s[:, m:m + 1])
                    qps = plg.tile([128, 512], F32, tag="lg")
                    for kk in range(4):
                        nc.tensor.matmul(qps[:, :nr],
                                         pT_s[:, kk * RANK:(kk + 1) * RANK],
                                         hpT[:, kk * 128:kk * 128 + nr],
                                         start=(kk == 0), stop=(kk == 3))
                    qeT = sb.tile([RANK, 128], BF16, tag=f"qeT{rt}", bufs=1)
                    nc.scalar.copy(qeT[:, :nr], qps[:, :nr])
                    qef = sb.tile([RANK, 128], F32, tag="qef")
                    nc.scalar.copy(qef[:, :nr], qps[:, :nr])
                    qtp = plg.tile([128, 512], F32, tag="lg")
                    nc.tensor.transpose(qtp[:nr, :RANK], qef[:, :nr], id4[:, :])
                    qe = sb.tile([128, RANK], F32, tag="qe")
                    nc.scalar.copy(qe[:nr, :], qtp[:nr, :RANK])
                    lt = sb.tile([128, RANK], F32, tag="lt")
                    nc.vector.tensor_mul(lt[:nr, :], qe[:nr, :],
                                         elab[:nr, rt * RANK:(rt + 1) * RANK])
                    nc.vector.reduce_sum(lab_ll[:nr, rt:rt + 1], lt[:nr, :],
                                         axis=AX.X)
                    return qeT

                def ce_unit(rt, qeT, nr, off, vc, first):
                    et = ebuf.tile([RANK, 512], BF16, tag="et")
                    nc.sync.dma_start(et[:, :vc], eT[:, off:off + vc])
                    ps = plg.tile([128, 512], F32, tag="lg")
                    nc.tensor.matmul(ps[:nr, :vc], qeT[:, :nr], et[:, :vc],
                                     start=True, stop=True)
                    ex = sb.tile([128, 512], BF16, tag="ex")
                    pexp = sb.tile([128, 1], F32, tag="pexp")
                    nc.scalar.activation(ex[:nr, :vc], ps[:nr, :vc], AF.Exp,
                                         accum_out=pexp[:nr, :])
                    if first:
                        nc.vector.tensor_copy(sum_e[:nr, rt:rt + 1], pexp[:nr, :])
                    else:
                        nc.vector.tensor_add(sum_e[:nr, rt:rt + 1],
                                             sum_e[:nr, rt:rt + 1], pexp[:nr, :])

                # CE worklist: rowtile 0 (128 rows) head at t=32, its vocab
                # chunks sprinkled ~5/step into the AG bubbles of steps 33+.
                HEAD_AT = {32: 0}
                qeTs = {}
                pending = []

                def g0_head(g0t, t):
                    nc.tensor.matmul(g0t[:], ones1[:], b01_s[:, 0:GS],
                                     start=True, stop=False)
                    for e in range(4):
                        nc.tensor.matmul(
                            g0t[:],
                            embT[:, e * NTOKP + t * B:e * NTOKP + (t + 1) * B],
                            w0e_s[:, e * GS:(e + 1) * GS],
                            start=False, stop=False)

                def g0_u0(g0t):
                    for ch in range(NCH):
                        nc.tensor.matmul(g0t[:], hT0[:, ch * B:(ch + 1) * B],
                                         u0_s[:, ch * GS:(ch + 1) * GS],
                                         start=False, stop=False)

                g0_cur = pg.tile([B, GS], F32, tag="g")
                g0_head(g0_cur, 0)
                g0_u0(g0_cur)

                for t in range(TS):
                    # ---- layer 0 (finish g0: ctx part) ----
                    for ch in range(NCH):
                        nc.tensor.matmul(g0_cur[:], ctxT[:, ch * B:(ch + 1) * B],
                                         w0c_s[:, ch * GS:(ch + 1) * GS],
                                         start=False, stop=(ch == NCH - 1))
                    h0n = lstm_layer(g0_cur, c0, 0)
                    gather_h(h0n, hT0, 0)

                    # g0(t+1) bias+emb fills the h0-AllGather window
                    g0_nxt = None
                    if t + 1 < TS:
                        g0_nxt = pg.tile([B, GS], F32, tag="g")
                        g0_head(g0_nxt, t + 1)

                    # ---- layer 1 ----
                    g1 = pg.tile([B, GS], F32, tag="g")
                    nc.tensor.matmul(g1[:], ones1[:], b01_s[:, GS:2 * GS],
                                     start=True, stop=False)
                    for ch in range(NCH):
                        nc.tensor.matmul(g1[:], hT1[:, ch * B:(ch + 1) * B],
                                         u1_s[:, ch * GS:(ch + 1) * GS],
                                         start=False, stop=False)
                    for ch in range(NCH):
                        nc.tensor.matmul(g1[:], hT0[:, ch * B:(ch + 1) * B],
                                         w1_s[:, ch * GS:(ch + 1) * GS],
                                         start=False, stop=(ch == NCH - 1))
                    h1n = lstm_layer(g1, c1, 1)
                    bo1 = gather_h(h1n, hT1, 1)

                    # g0(t+1) U0 part fills the h1-AllGather window
                    if g0_nxt is not None:
                        g0_u0(g0_nxt)

                    # own-batch columns straight from the collective output
                    nc.gpsimd.dma_start(
                        hT1own[:].rearrange("p (c o j) -> p c o j",
                                            o=1, j=BS),
                        bo1[:].rearrange("(c p) (k j) -> p c k j",
                                         p=128, j=BS)[
                            :, :, bass.ds(pid, 1), :])
                    for ch in range(NCH):
                        nc.vector.tensor_copy(
                            featsT[:, ch * NROW + t * BS:
                                   ch * NROW + (t + 1) * BS],
                            hT1own[:, ch * BS:(ch + 1) * BS])

                    # ---- attention (own batches, batched softmax) ----
                    sc = psc.tile([128, S], F32, tag="sc")
                    for j in range(BS):
                        for ch in range(NCH):
                            nc.tensor.matmul(
                                sc[32 * j:32 * j + 1, :],
                                hT1own[:, ch * BS + j:ch * BS + j + 1],
                                encT_s[:, (ch * BS + j) * S:
                                       (ch * BS + j + 1) * S],
                                start=(ch == 0), stop=(ch == NCH - 1),
                                tile_position=(0, 32 * j))
                    nc.vector.tensor_add(scmt[:], sc[:], smask_s[:])
                    mx = sb.tile([128, 1], F32, tag="mx")
                    nc.vector.reduce_max(mx[:], scmt[:], axis=AX.X)
                    nmx = sb.tile([128, 1], F32, tag="nmx")
                    nc.vector.tensor_scalar_mul(nmx[:], mx[:], -1.0)
                    ssum = sb.tile([128, 1], F32, tag="ssum")
                    pe_ = sb.tile([128, S], F32, tag="pe")
                    nc.scalar.activation(pe_[:], scmt[:], AF.Exp,
                                         bias=nmx[:, :], accum_out=ssum[:])
                    rs = sb.tile([128, 1], F32, tag="rs")
                    nc.vector.reciprocal(rs[:], ssum[:])
                    pbf = sb.tile([128, S], F32, tag="pbf")
                    nc.vector.tensor_scalar_mul(pbf[:], pe_[:], rs[:, :])
                    ptp_ = pptp.tile([S, 128], F32, tag="ptp")
                    nc.tensor.transpose(ptp_[:], pbf[:], id4[:, :])
                    pTs = sb.tile([S, 128], BF16, tag="pTs")
                    nc.scalar.copy(pTs[:], ptp_[:])

                    ctxo = pcx.tile([128, NCH * BS], F32, tag="ctxo")
                    for j in range(BS):
                        for ch in range(NCH):
                            nc.tensor.matmul(
                                ctxo[:, ch * BS + j:ch * BS + j + 1],
                                enc_s[:, j * C + ch * 128:
                                      j * C + (ch + 1) * 128],
                                pTs[:, 32 * j:32 * j + 1],
                                start=True, stop=True)

                    ctxTo = sb.tile([128, NCH * BS], BF16, tag="ctxTo")
                    nc.scalar.copy(ctxTo[:], ctxo[:])
                    for ch in range(NCH):
                        nc.vector.tensor_copy(
                            featsT[:, (NCH + ch) * NROW + t * BS:
                                   (NCH + ch) * NROW + (t + 1) * BS],
                            ctxTo[:, ch * BS:(ch + 1) * BS])
                    if t + 1 < TS:
                        bi = dram.tile([128, NCH * BS], BF16, tag="agi2")
                        nc.sync.dma_start(bi[:], ctxTo[:])
                        bo = dram.tile([128 * NCORES, NCH * BS], BF16, tag="ago2")
                        nc.gpsimd.collective_compute(
                            "AllGather", ALU.bypass, replica_groups=rg,
                            ins=[bi.opt()], outs=[bo.opt()])
                        nc.sync.dma_start(
                            ctxKin[:].rearrange("p (k cj) -> p k cj", k=NCORES),
                            bo[:].rearrange("(k p) cj -> p k cj", p=128))
                        nc.vector.tensor_copy(
                            ctxT[:].rearrange("p (c k j) -> p k c j",
                                              c=NCH, k=NCORES),
                            ctxKin[:].rearrange("p (k c j) -> p k c j",
                                                k=NCORES, c=NCH))

                    # ---- interleaved CE work ----
                    if t in HEAD_AT:
                        rt = HEAD_AT[t]
                        qeTs[rt] = ce_head(rt, 0, 128)
                        off = 0
                        for ui, vc in enumerate(VCHUNKS):
                            pending.append((rt, off, vc, ui == 0))
                            off += vc
                    for _ in range(5):
                        if pending:
                            rt, o_, v_, fr = pending.pop(0)
                            ce_unit(rt, qeTs[rt], 128, o_, v_, fr)

                    g0_cur = g0_nxt

                for rt, o_, v_, fr in pending:
                    ce_unit(rt, qeTs[rt], 128, o_, v_, fr)
                qeTs[1] = ce_head(1, 128, NROW - 128)
                off = 0
                for ui, vc in enumerate(VCHUNKS):
                    ce_unit(1, qeTs[1], NROW - 128, off, vc, ui == 0)
                    off += vc

                # ---- finalize loss ----
                lse = sb.tile([128, 2], F32, tag="lse")
                nc.scalar.activation(lse[:], sum_e[:], AF.Ln)
                nll = sb.tile([128, 2], F32, tag="nll")
                nc.vector.tensor_sub(nll[:], lse[:], lab_ll[:])
                nllm = sb.tile([128, 2], DT4, tag="nllm")
                nc.vector.tensor_mul(nllm[:], nll[:], vmask_s[:])
                lp = pptp.tile([1, 2], F32, tag="ptp")
                nc.tensor.matmul(lp[:], ones128[:, :], nllm[:],
                                 start=True, stop=True)
                lsum = sb.tile([1, 1], F32, tag="lsum")
                nc.vector.reduce_sum(lsum[:], lp[:], axis=AX.X)
                lbi = dram.tile([1, 1], F32, tag="lbi")
                nc.sync.dma_start(lbi[:], lsum[:])
                lbo = dram.tile([1, 1], F32, tag="lbo")
                nc.gpsimd.collective_compute(
                    "AllReduce", ALU.add, replica_groups=rg,
                    ins=[lbi.opt()], outs=[lbo.opt()])
                nc.sync.dma_start(out_loss[:], lbo[:])

    nc.compile()
    return nc


def _prep_inputs(inputs):
    f32 = np.float32
    enc = np.asarray(inputs["encoded"], f32)
    est = np.asarray(inputs["encoder_state"], f32)
    tok = np.asarray(inputs["tgt_tokens"]).astype(np.int32)
    enc_lens = np.asarray(inputs["enc_lens"]).astype(np.int32)
    tgt_lens = np.asarray(inputs["tgt_lens"]).astype(np.int32)
    E = np.asarray(inputs["E"], f32)
    P = np.asarray(inputs["P"], f32)
    W0 = np.asarray(inputs["W0"], f32)
    U0 = np.asarray(inputs["U0"], f32)
    b0 = np.asarray(inputs["b0"], f32)
    W1 = np.asarray(inputs["W1"], f32)
    U1 = np.asarray(inputs["U1"], f32)
    b1 = np.asarray(inputs["b1"], f32)
    Wo = np.asarray(inputs["Wo"], f32)
    bo = np.asarray(inputs["bo"], f32)

    encT = np.ascontiguousarray(enc.transpose(2, 1, 0))       # [C, B, S]
    hT_init = np.ascontiguousarray(
        est.transpose(0, 2, 1).reshape(2 * H, B)).astype(BF16NP)
    eT = np.ascontiguousarray(E.T).astype(BF16NP)
    tokidx = np.zeros((NTOKP, 1), np.int32)
    tokidx[:NTOK, 0] = tok[:, :TS].T.reshape(NTOK)
    pTb = np.ascontiguousarray(P.T).astype(BF16NP)
    wo_b = Wo.astype(BF16NP)
    boT = np.ascontiguousarray(bo.reshape(4, 128).T)          # [128, 4]

    ones_c = np.ones((128, B + 1), f32)
    zctx_c = np.zeros((128, NCH * B), BF16NP)
    in_maps = []
    for k in range(NCORES):
        cols = np.concatenate(
            [np.arange(g * H + k * HS, g * H + (k + 1) * HS)
             for g in range(4)])
        ob = slice(k * BS, (k + 1) * BS)
        enc_o = np.ascontiguousarray(
            enc[:, ob, :].reshape(S, BS * C)).astype(BF16NP)
        encT_o = np.ascontiguousarray(
            encT[:, ob, :].reshape(NCH, 128, BS * S)
            .transpose(1, 0, 2).reshape(128, NCH * BS * S)).astype(BF16NP)
        sm = np.zeros((128, S), f32)
        for j in range(BS):
            sm[32 * j, :] = np.where(
                np.arange(S) >= enc_lens[k * BS + j], f32(-1e30), f32(0.0))
        lab = tok[ob, 1:T].T.reshape(NROW, 1)
        vm = (np.arange(TS)[:, None] <
              (tgt_lens[ob] - 1)[None, :]).astype(f32).reshape(NROW)
        vmp = np.zeros((128, 2), f32)
        vmp[:128, 0] = vm[:128]
        vmp[:NROW - 128, 1] = vm[128:]
        in_maps.append({
            "w0e": np.ascontiguousarray(W0[:EMB, cols]).astype(BF16NP),
            "w0c": np.ascontiguousarray(W0[EMB:, cols]).astype(BF16NP),
            "u0": np.ascontiguousarray(U0[:, cols]).astype(BF16NP),
            "w1": np.ascontiguousarray(W1[:, cols]).astype(BF16NP),
            "u1": np.ascontiguousarray(U1[:, cols]).astype(BF16NP),
            "b01": np.ascontiguousarray(np.concatenate([b0[cols], b1[cols]]).reshape(1, 2 * GS)),
            "wo": wo_b,
            "boT": boT,
            "pmat": P,
            "pTb": pTb,
            "enc_own": enc_o,
            "encT_own": encT_o,
            "smask": sm,
            "hT_init": hT_init,
            "e_rows": E,
            "eT": eT,
            "tokidx": tokidx,
            "labidx": np.ascontiguousarray(lab.astype(np.int32)),
            "vmask": vmp,
            "ones_in": ones_c,
            "zctx": zctx_c,
        })
    return in_maps


_NC_CACHE = {}


def kernel(**inputs) -> np.ndarray:
    if "nc" not in _NC_CACHE:
        _NC_CACHE["nc"] = build()
    nc = _NC_CACHE["nc"]
    in_maps = _prep_inputs(inputs)
    res = bass_utils.run_bass_kernel_spmd(
        nc, in_maps, core_ids=list(range(NCORES)))
    _NC_CACHE["res"] = res
    return np.float32(res.results[0]["loss"][0, 0])
